# revision 29
# baseline (speedup 1.0000x reference)
"""Trainium2 Bass kernel for nn_AdaptBlockV2 (deformable-conv-v2 block).

Data-parallel over the batch axis: 8 samples -> 8 NeuronCores, one sample
per core. Inside each core:
  A) load x; build zero-padded CHW copy (bf16) for the convs; transpose x to
     HWC and write a "quad" gather table to DRAM (row r = channels of flat
     pixels [r, r+1, r+W, r+W+1], bf16) -- one indirect-DMA descriptor then
     fetches all 4 bilinear corners of one (pixel, tap).
  B) 15-channel 3x3 conv (offset transform T, translation tr, modulation
     mask) as 9 PSUM-accumulated matmuls; transpose conv output to
     pixel-major; bulk DVE math for sampling positions py/px, floor via
     floored-mod, corner weights (bilinear x mask x validity), and the flat
     gather index.
  C) per-slice pipeline: indirect DMA gather -> DVE weighted 4-corner
     combine -> PE transpose of samp to (tap,channel)-major -> matmul with
     dw -> BN (running stats) + residual + ReLU epilogue -> DMA out.

kernel(**inputs) takes FULL unsharded inputs, returns the FULL output.
"""
import numpy as np
import ml_dtypes

N, C, H, W = 8, 48, 96, 72
HW = H * W                       # 6912
LEAD = W + 2                     # 74: lead pad rows in the quad table
RQ = 7040                        # quad-table rows (55*128; >= HW+W+2)
QW = 256                         # quad-table row width (512B, dma_gather)
NB = HW // 128                   # 54 pixel blocks
QTOT = NB * 9                    # 486 (block, tap) chunks
PADW = W + 2                     # 74 padded conv row stride
PADLEN = (H + 2) * PADW          # 7252
BN_EPS = 1e-5
CONV_ROWS = 7                    # conv N-tile = 7 image rows = 504 pixels
SLICE_BLOCKS = 4                 # gather/combine slice = 4 pixel blocks

_REG = np.array([[-1, -1, -1, 0, 0, 0, 1, 1, 1],
                 [-1, 0, 1, -1, 0, 1, -1, 0, 1]], dtype=np.float32)

_built = {}


def _slices():
    out = []
    b = 0
    while b < NB:
        nb = min(SLICE_BLOCKS, NB - b)
        out.append((b, nb))
        b += nb
    return out


def build_nc(debug_taps=False, max_slices=None, stage='full'):
    import concourse.bass as bass
    import concourse.bacc as bacc
    import concourse.tile as tile
    from concourse import mybir
    from concourse.bass import IndirectOffsetOnAxis, AP
    from concourse.masks import make_identity
    from concourse.tile import add_dep_helper
    from contextlib import ExitStack

    dt = mybir.dt
    op = mybir.AluOpType
    act = mybir.ActivationFunctionType

    nc = bacc.Bacc("TRN2", target_bir_lowering=False, debug=False,
                   num_devices=N, dynamic_dma_scratch_size=16384)
    x_ext = nc.declare_dram_parameter("x", [C, HW], dt.float32, isOutput=False)
    wconv_ext = nc.declare_dram_parameter("wconv", [C, 135], dt.bfloat16, isOutput=False)
    bconv_ext = nc.declare_dram_parameter("bconv", [15], dt.float32, isOutput=False)
    dwt_ext = nc.declare_dram_parameter("dwt", [128, 192], dt.bfloat16, isOutput=False)
    reg0_ext = nc.declare_dram_parameter("reg0", [128, 9], dt.float32, isOutput=False)
    reg1_ext = nc.declare_dram_parameter("reg1", [128, 9], dt.float32, isOutput=False)
    yc_ext = nc.declare_dram_parameter("ycoord", [128, NB], dt.float32, isOutput=False)
    xc_ext = nc.declare_dram_parameter("xcoord", [128, NB], dt.float32, isOutput=False)
    gamma_ext = nc.declare_dram_parameter("gamma", [C], dt.float32, isOutput=False)
    beta_ext = nc.declare_dram_parameter("beta", [C], dt.float32, isOutput=False)
    rmean_ext = nc.declare_dram_parameter("rmean", [C], dt.float32, isOutput=False)
    rvar_ext = nc.declare_dram_parameter("rvar", [C], dt.float32, isOutput=False)
    out_ext = nc.declare_dram_parameter("out", [C, HW], dt.float32, isOutput=True)
    dbg = {}
    if debug_taps:
        for nm, shape, dty in (
                ("dbg_tcols", [128, NB * 15], dt.float32),
                ("dbg_idx", [128, QTOT], dt.int16),
                ("dbg_w4", [128, QTOT * 4], dt.bfloat16),
                ("dbg_gq", [128, SLICE_BLOCKS * 9 * QW], dt.bfloat16),
                ("dbg_samp", [128, SLICE_BLOCKS * 9 * C], dt.bfloat16),
                ("dbg_sampt", [128, 3 * 512], dt.bfloat16),
                ("dbg_hwc", [128, NB * C], dt.bfloat16),
                ("dbg_xq", [128, QW], dt.bfloat16),
                ("dbg_idxw", [128, QTOT * 8], dt.int16)):
            dbg[nm] = nc.declare_dram_parameter(nm, shape, dty, isOutput=True)

    x_quad = nc.dram_tensor("x_quad", [RQ, QW], dt.bfloat16)
    idx_dram = nc.dram_tensor("idx_dram", [128 * QTOT], dt.int16)

    with tile.TileContext(nc) as tc, ExitStack() as ctx:
        cp = ctx.enter_context(tc.tile_pool(name="const", bufs=1))
        tp = ctx.enter_context(tc.tile_pool(name="tmp", bufs=1))
        wp = ctx.enter_context(tc.tile_pool(name="work", bufs=2))
        pp_a = ctx.enter_context(tc.tile_pool(name="ps_a", bufs=2, space="PSUM"))
        pp_st = ctx.enter_context(tc.tile_pool(name="ps_st", bufs=2, space="PSUM"))
        pp_out = ctx.enter_context(tc.tile_pool(name="ps_out", bufs=2, space="PSUM"))

        # ---------------- constants / weights to SBUF ----------------
        x_sb = cp.tile([C, HW], dt.float32, tag="x_sb")
        nc.sync.dma_start(x_sb[:], x_ext[:])
        wconv_t = cp.tile([C, 135], dt.bfloat16, tag="wconv")
        nc.sync.dma_start(wconv_t[:], wconv_ext[:])
        bconv_t = cp.tile([15, 1], dt.float32, tag="bconv")
        nc.sync.dma_start(bconv_t[:], bconv_ext[:])
        dwt_t = cp.tile([128, 192], dt.bfloat16, tag="dwt")
        nc.sync.dma_start(dwt_t[:], dwt_ext[:])
        reg0_t = cp.tile([128, 9], dt.float32, tag="reg0")
        nc.sync.dma_start(reg0_t[:], reg0_ext[:])
        reg1_t = cp.tile([128, 9], dt.float32, tag="reg1")
        nc.sync.dma_start(reg1_t[:], reg1_ext[:])
        yc_t = cp.tile([128, NB], dt.float32, tag="yc")
        nc.sync.dma_start(yc_t[:], yc_ext[:])
        xc_t = cp.tile([128, NB], dt.float32, tag="xc")
        nc.sync.dma_start(xc_t[:], xc_ext[:])

        bn_in = {}
        for nm, ext in (("gamma", gamma_ext), ("beta", beta_ext),
                        ("rmean", rmean_ext), ("rvar", rvar_ext)):
            t = cp.tile([C, 1], dt.float32, tag=nm)
            nc.sync.dma_start(t[:], ext[:])
            bn_in[nm] = t

        id48 = cp.tile([C, C], dt.float32, tag="id48")
        make_identity(nc, id48[:])
        id16 = id48[0:15, 0:15]
        id128 = cp.tile([128, 128], dt.bfloat16, tag="id128")
        make_identity(nc, id128[:])

        # bn scale' = gamma * rsqrt(rvar+eps); shift' = beta - rmean*scale'
        veps = tp.tile([C, 1], dt.float32, tag="veps")
        nc.vector.tensor_scalar(veps[:], bn_in["rvar"][:], BN_EPS, None, op.add)
        vsq = tp.tile([C, 1], dt.float32, tag="vsq")
        nc.scalar.activation(vsq[:], veps[:], act.Sqrt)
        vri = tp.tile([C, 1], dt.float32, tag="vri")
        nc.vector.reciprocal(vri[:], vsq[:])
        scale_t = cp.tile([C, 1], dt.float32, tag="scale")
        nc.vector.tensor_tensor(scale_t[:], bn_in["gamma"][:], vri[:], op.mult)
        vms = tp.tile([C, 1], dt.float32, tag="vms")
        nc.vector.tensor_tensor(vms[:], bn_in["rmean"][:], scale_t[:], op.mult)
        shift_t = cp.tile([C, 1], dt.float32, tag="shift")
        nc.vector.tensor_tensor(shift_t[:], bn_in["beta"][:], vms[:], op.subtract)

        # ---------------- padded CHW copy (bf16) for convs ----------------
        x_pad = cp.tile([C, PADLEN], dt.bfloat16, tag="x_pad")
        nc.vector.memset(x_pad[:], 0.0)
        xpad_int = AP(x_pad.tensor, x_pad[:].offset + PADW + 1,
                      [x_pad[:].ap[0], [PADW, H], [1, W]])
        nc.vector.tensor_copy(xpad_int, x_sb[:])   # f32 -> bf16 cast on DVE

        # ---------------- x -> HWC (bf16) via PE transposes ----------------
        x_hwc = cp.tile([128, NB * C], dt.bfloat16, tag="x_hwc")
        for g in range((NB + 3) // 4):          # 4 blocks per PSUM tile
            nblk = min(4, NB - g * 4)
            ps = pp_a.tile([128, 4 * C], dt.float32, name="psA", tag="psA")
            for j in range(nblk):
                b = g * 4 + j
                nc.tensor.transpose(ps[:, j * C:(j + 1) * C],
                                    x_sb[:, b * 128:(b + 1) * 128], id48[:])
            nc.scalar.activation(x_hwc[:, g * 4 * C:(g * 4 + nblk) * C],
                                 ps[:, :nblk * C], act.Copy)

        # ---------------- quad table to DRAM ----------------
        # Zero the whole table (5 chained big writes), then write each slot
        # column j = x_hwc at row offset LEAD-shift_j. Issue chain keeps
        # order; gathers sem-wait on the last write only.
        zsrc = cp.tile([128, 1408], dt.bfloat16, tag="zsrc")
        nc.vector.memset(zsrc[:], 0.0)
        chain = []
        for zi in range(10):                     # 10 * 704 rows = 7040
            dst = AP(x_quad, zi * 704 * QW,
                     [[1408, 128], [1, 1408]])
            chain.append(nc.sync.dma_start(out=dst, in_=zsrc[:]))
        for j, shift in enumerate((0, 1, W, W + 1)):
            dst = AP(x_quad, (LEAD - shift) * QW + j * 64,
                     [[QW, 128], [128 * QW, NB], [1, C]])
            src = AP(x_hwc.tensor, x_hwc[:].offset,
                     [x_hwc[:].ap[0], [C, NB], [1, C]])
            chain.append(nc.sync.dma_start(out=dst, in_=src))
        for a, b in zip(chain[1:], chain[:-1]):
            add_dep_helper(a.ins, b.ins, sync=False,
                           reason="quad-table write chain")
        quad_writes = [chain[-1]]

        # ---------------- convs: 15ch 3x3 via 9 accumulated matmuls --------
        conv_sb = cp.tile([15, HW], dt.float32, tag="conv_sb")
        trows = [(t * CONV_ROWS, min(CONV_ROWS, H - t * CONV_ROWS))
                 for t in range((H + CONV_ROWS - 1) // CONV_ROWS)]
        for (r0, nr) in trows:
            psc = pp_a.tile([15, CONV_ROWS * W], dt.float32, name="psA2", tag="psA2")
            npx = nr * W
            for s in range(9):
                r, c = divmod(s, 3)
                rhs = AP(x_pad.tensor,
                         x_pad[:].offset + (r0 + r) * PADW + c,
                         [x_pad[:].ap[0], [PADW, nr], [1, W]])
                nc.tensor.matmul(psc[:, :npx], wconv_t[:, s * 15:(s + 1) * 15],
                                 rhs, start=(s == 0), stop=(s == 8))
            nc.scalar.activation(conv_sb[:, r0 * W:r0 * W + npx], psc[:, :npx],
                                 act.Identity, bias=bconv_t[:])

        # ---------------- conv output -> pixel-major (tcols) ---------------
        tcols = cp.tile([128, NB * 15], dt.float32, tag="tcols")
        for g in range((NB + 7) // 8):          # 8 blocks per PSUM tile
            nblk = min(8, NB - g * 8)
            ps = pp_a.tile([128, 8 * 15], dt.float32, name="psA", tag="psA")
            for j in range(nblk):
                b = g * 8 + j
                nc.tensor.transpose(ps[:, j * 15:(j + 1) * 15],
                                    conv_sb[:, b * 128:(b + 1) * 128],
                                    id16[:])
            nc.scalar.activation(tcols[:, g * 8 * 15:(g * 8 + nblk) * 15],
                                 ps[:, :nblk * 15], act.Copy)

        # ---------------- bulk offset / weight / index math ----------------
        def plane(tag):
            return tp.tile([128, QTOT], dt.float32, name=tag, tag=tag)

        def tcol_b(ch):     # tcols channel ch broadcast over taps [128,NB,9]
            return AP(tcols.tensor, tcols[:].offset + ch,
                      [tcols[:].ap[0], [15, NB], [0, 9]])

        def reg_b(t):       # REG row broadcast over blocks
            return AP(t.tensor, t[:].offset, [t[:].ap[0], [0, NB], [1, 9]])

        def coord_b(t):     # per-pixel coord broadcast over taps
            return AP(t.tensor, t[:].offset, [t[:].ap[0], [1, NB], [0, 9]])

        py = plane("py"); px = plane("px")
        t1 = plane("t1"); t2 = plane("t2"); t3 = plane("t3")
        fy = plane("fy"); fx = plane("fx")
        y0 = plane("y0"); x0 = plane("x0")
        wy0 = py; wy1 = px                       # reuse dead slots (disjoint cols)
        wx0 = plane("wx0"); wx1 = plane("wx1")
        idxf = fy                                # reuse dead slot
        cvt_i = tp.tile([128, QTOT], dt.int32, name="cvt_i", tag="cvt_i")
        w4 = cp.tile([128, QTOT * 4], dt.bfloat16, tag="w4")
        idx_t = cp.tile([128, QTOT], dt.int16, tag="idx")
        fold = cp.tile([16, 8 * QTOT], dt.int16, tag="fold")
        idxw = cp.tile([128, QTOT * 8], dt.int16, tag="idxw")

        # offset math emitted in groups of blocks so the first gathers can
        # start while later groups' index math is still running
        def emit_group(g0, gn):
            nq = gn * 9
            c0 = g0 * 9
            sl = slice(c0, c0 + nq)

            def tcol_b(ch):
                return AP(tcols.tensor, tcols[:].offset + ch + g0 * 15,
                          [tcols[:].ap[0], [15, gn], [0, 9]])

            def reg_b(t):
                return AP(t.tensor, t[:].offset, [t[:].ap[0], [0, gn], [1, 9]])

            def coord_b(t):
                return AP(t.tensor, t[:].offset + g0,
                          [t[:].ap[0], [1, gn], [0, 9]])

            tt = nc.vector.tensor_tensor
            ts = nc.vector.tensor_scalar
            tt(t1[:, sl], reg_b(reg0_t), tcol_b(0), op.mult)
            tt(t2[:, sl], reg_b(reg1_t), tcol_b(1), op.mult)
            tt(t3[:, sl], t1[:, sl], t2[:, sl], op.add)
            tt(t1[:, sl], t3[:, sl], tcol_b(4), op.add)
            tt(py[:, sl], t1[:, sl], coord_b(yc_t), op.add)
            tt(t1[:, sl], reg_b(reg0_t), tcol_b(2), op.mult)
            tt(t2[:, sl], reg_b(reg1_t), tcol_b(3), op.mult)
            tt(t3[:, sl], t1[:, sl], t2[:, sl], op.add)
            tt(t1[:, sl], t3[:, sl], tcol_b(5), op.add)
            tt(px[:, sl], t1[:, sl], coord_b(xc_t), op.add)

            # floor(v) = int(v) - (v < int(v)); rounding-mode agnostic
            for (v, fl, fr) in ((py, y0, fy), (px, x0, fx)):
                nc.vector.tensor_copy(cvt_i[:, sl], v[:, sl])
                nc.vector.tensor_copy(t1[:, sl], cvt_i[:, sl])
                tt(t2[:, sl], v[:, sl], t1[:, sl], op.is_lt)
                tt(fl[:, sl], t1[:, sl], t2[:, sl], op.subtract)
                tt(fr[:, sl], v[:, sl], fl[:, sl], op.subtract)

            mk_b = AP(tcols.tensor, tcols[:].offset + 6 + g0 * 15,
                      [tcols[:].ap[0], [15, gn], [1, 9]])
            ts(t1[:, sl], y0[:, sl], 0.0, None, op.is_ge)
            ts(t2[:, sl], y0[:, sl], float(H - 1), None, op.is_le)
            tt(t3[:, sl], t1[:, sl], t2[:, sl], op.mult)
            ts(t1[:, sl], fy[:, sl], -1.0, 1.0, op.mult, op.add)
            tt(wy0[:, sl], t1[:, sl], t3[:, sl], op.mult)
            ts(t1[:, sl], y0[:, sl], -1.0, None, op.is_ge)
            ts(t2[:, sl], y0[:, sl], float(H - 2), None, op.is_le)
            tt(t3[:, sl], t1[:, sl], t2[:, sl], op.mult)
            tt(wy1[:, sl], fy[:, sl], t3[:, sl], op.mult)
            ts(t1[:, sl], x0[:, sl], 0.0, None, op.is_ge)
            ts(t2[:, sl], x0[:, sl], float(W - 1), None, op.is_le)
            tt(t3[:, sl], t1[:, sl], t2[:, sl], op.mult)
            ts(t1[:, sl], fx[:, sl], -1.0, 1.0, op.mult, op.add)
            tt(wx0[:, sl], t1[:, sl], t3[:, sl], op.mult)
            ts(t1[:, sl], x0[:, sl], -1.0, None, op.is_ge)
            ts(t2[:, sl], x0[:, sl], float(W - 2), None, op.is_le)
            tt(t3[:, sl], t1[:, sl], t2[:, sl], op.mult)
            tt(wx1[:, sl], fx[:, sl], t3[:, sl], op.mult)
            tt(wy0[:, sl], wy0[:, sl], mk_b, op.mult)
            tt(wy1[:, sl], wy1[:, sl], mk_b, op.mult)

            def w4_slot(j):
                return AP(w4.tensor, w4[:].offset + j + c0 * 4,
                          [w4[:].ap[0], [4, nq]])
            tt(w4_slot(0), wy0[:, sl], wx0[:, sl], op.mult)
            tt(w4_slot(1), wy0[:, sl], wx1[:, sl], op.mult)
            tt(w4_slot(2), wy1[:, sl], wx0[:, sl], op.mult)
            tt(w4_slot(3), wy1[:, sl], wx1[:, sl], op.mult)

            # gather index = clip(LEAD + y0*W + x0, 0, HW+W+1), int16
            nc.vector.scalar_tensor_tensor(idxf[:, sl], y0[:, sl], float(W),
                                           x0[:, sl], op.mult, op.add)
            ts(idxf[:, sl], idxf[:, sl], float(LEAD), 0.0, op.add, op.max)
            ts(idxf[:, sl], idxf[:, sl], float(HW + W + 1), None, op.min)
            nc.vector.tensor_copy(idx_t[:, sl], idxf[:, sl])

            # 16-wrap via DRAM round-trip fold + in-partition interleave
            wr = nc.sync.dma_start(
                out=AP(idx_dram, c0, [[QTOT, 128], [1, nq]]),
                in_=idx_t[:, sl])
            rdf = nc.sync.dma_start(
                out=AP(fold.tensor, fold[:].offset + c0,
                       [fold[:].ap[0], [QTOT, 8], [1, nq]]),
                in_=AP(idx_dram, c0, [[QTOT, 16], [16 * QTOT, 8], [1, nq]]))
            add_dep_helper(rdf.ins, wr.ins, sync=False, reason="idx fold rt")
            i16 = idxw[0:16, :]
            nc.vector.tensor_copy(
                AP(i16.tensor, i16.offset + c0 * 8,
                   [i16.ap[0], [1, 8], [8, nq]]),
                AP(fold.tensor, fold[:].offset + c0,
                   [fold[:].ap[0], [QTOT, 8], [1, nq]]))
            for g in range(1, 8):
                nc.sync.dma_start(
                    out=idxw[16 * g:16 * (g + 1), c0 * 8:(c0 + nq) * 8],
                    in_=idxw[0:16, c0 * 8:(c0 + nq) * 8])

        for (g0, gn) in ((0, 9), (9, 9), (18, 18), (36, 18)):
            emit_group(g0, gn)

        # ---------------- gather / combine / matmul / epilogue -------------
        if debug_taps:
            nc.sync.dma_start(dbg["dbg_tcols"][:], tcols[:])
            nc.sync.dma_start(dbg["dbg_idx"][:], idx_t[:])
            nc.sync.dma_start(dbg["dbg_w4"][:], w4[:])
            nc.sync.dma_start(dbg["dbg_hwc"][:], x_hwc[:])
            xqs = cp.tile([128, QW], dt.bfloat16, tag="xqs")
            rd = nc.sync.dma_start(out=xqs[:], in_=x_quad[100:228, :])
            add_dep_helper(rd.ins, quad_writes[0].ins,
                           reason="dbg read after quad build")
            nc.sync.dma_start(dbg["dbg_xq"][:], xqs[:])
            wdep = nc.sync.dma_start(dbg["dbg_idxw"][:], idxw[:])

        slist = _slices() if max_slices is None else _slices()[:max_slices]
        for si, (b0, nb) in enumerate(slist):
            Q = nb * 9
            npx = nb * 128
            q0 = b0 * 9
            gq = wp.tile([128, SLICE_BLOCKS * 9 * QW], dt.bfloat16, tag="gq",
                         bufs=3)
            gq3 = AP(gq.tensor, gq[:].offset,
                     [gq[:].ap[0], [QW, Q], [1, QW]])
            gi = nc.gpsimd.dma_gather(
                out_ap=gq3,
                in_ap=x_quad[:],
                idxs_ap=idxw[:, q0 * 8:(q0 + Q) * 8],
                num_idxs=Q * 128,
                num_idxs_reg=Q * 128,
                elem_size=QW,
                single_packet=False,
            )
            for qw in quad_writes:
                add_dep_helper(gi.ins, qw.ins, reason="gather after quad build")

            if debug_taps and si == 0:
                nc.sync.dma_start(dbg["dbg_gq"][:], gq[:])

            if stage == 'gather':
                continue
            # weighted 4-corner combine (in-place over the gathered tile;
            # slot pads multiply garbage that is never read)
            prod = gq
            gview = gq[:, :Q * QW]
            wb = AP(w4.tensor, w4[:].offset + q0 * 4,
                    [w4[:].ap[0], [1, Q * 4], [0, 64]])
            nc.vector.tensor_tensor(gview, gview, wb, op.mult)

            def pview(off):
                return AP(prod.tensor, prod[:].offset + off,
                          [prod[:].ap[0], [QW, Q], [1, C]])
            sa = wp.tile([128, SLICE_BLOCKS * 9 * C], dt.bfloat16, tag="sa")
            sb2 = wp.tile([128, SLICE_BLOCKS * 9 * C], dt.bfloat16, tag="sb2")
            nc.vector.tensor_tensor(sa[:, :Q * C], pview(0), pview(64), op.add)
            nc.vector.tensor_tensor(sb2[:, :Q * C], pview(128), pview(192),
                                    op.add)
            samp = sa
            nc.vector.tensor_tensor(samp[:, :Q * C], sa[:, :Q * C],
                                    sb2[:, :Q * C], op.add)

            if debug_taps and si == 0:
                nc.sync.dma_start(dbg["dbg_samp"][:], samp[:])

            if stage == 'combine':
                continue
            # transpose samp [128px, 432] -> sampt [(tap,ch) 128-chunks, px]
            sampt = wp.tile([128, 4 * SLICE_BLOCKS * 128], dt.bfloat16, tag="sampt")
            for ib in range(nb):
                ps = pp_st.tile([128, 512], dt.bfloat16, tag="ps_st")
                base = ib * 9 * C
                for ch in range(3):
                    nc.tensor.transpose(
                        ps[:, ch * 128:(ch + 1) * 128],
                        samp[:, base + ch * 128: base + (ch + 1) * 128],
                        id128[:])
                nc.tensor.transpose(ps[0:C, 384:512],
                                    samp[:, base + 384: base + 432], id128[:])
                dst02 = AP(sampt.tensor, sampt[:].offset + ib * 128,
                           [sampt[:].ap[0], [npx, 3], [1, 128]])
                nc.scalar.activation(dst02, ps[:, 0:384], act.Copy)
                nc.scalar.activation(sampt[0:C, 3 * npx + ib * 128:
                                           3 * npx + (ib + 1) * 128],
                                     ps[0:C, 384:512], act.Copy)

            if debug_taps and si == 0:
                nc.sync.dma_start(dbg["dbg_sampt"][:], sampt[:, 0:3 * 512])

            if stage == 'transpose':
                continue
            # matmul: out[o, px] += dwT_chunk.T @ sampt_chunk
            pso = pp_out.tile([C, SLICE_BLOCKS * 128], dt.float32, tag="ps_out")
            for ch in range(3):
                nc.tensor.matmul(pso[:, :npx], dwt_t[:, ch * C:(ch + 1) * C],
                                 sampt[:, ch * npx:(ch + 1) * npx],
                                 start=(ch == 0), stop=False)
            nc.tensor.matmul(pso[:, :npx], dwt_t[0:C, 3 * C:4 * C],
                             sampt[0:C, 3 * npx:3 * npx + npx],
                             start=False, stop=True)

            # epilogue: BN(running stats) + residual + relu -> DRAM
            bno = wp.tile([C, SLICE_BLOCKS * 128], dt.float32, tag="bno")
            nc.scalar.activation(bno[:, :npx], pso[:, :npx], act.Identity,
                                 bias=shift_t[:], scale=scale_t[:])
            nc.vector.tensor_tensor(bno[:, :npx], bno[:, :npx],
                                    x_sb[:, b0 * 128:b0 * 128 + npx], op.add)
            nc.vector.tensor_scalar(bno[:, :npx], bno[:, :npx], 0.0, None,
                                    op.max)
            nc.sync.dma_start(out_ext[:, b0 * 128:b0 * 128 + npx],
                              bno[:, :npx])

    nc.compile()
    return nc


def _host_pack(inputs):
    """Weight/constant layout prep (no input-data compute)."""
    bf16 = ml_dtypes.bfloat16
    wa = np.concatenate([inputs['tm_w'], inputs['tr_w'], inputs['mk_w']],
                        axis=0)                       # [15, C, 3, 3]
    wconv = np.ascontiguousarray(
        wa.transpose(1, 2, 3, 0).reshape(C, 9 * 15)).astype(bf16)
    bconv = np.concatenate([inputs['tm_b'], inputs['tr_b'],
                            inputs['mk_b']]).astype(np.float32)
    dwr = inputs['dw'].reshape(C, C, 9)               # [o, c, k]
    dwT = np.zeros((512, C), dtype=np.float32)
    dwT[:432] = dwr.transpose(2, 1, 0).reshape(9 * C, C)
    dwt = np.ascontiguousarray(
        dwT.reshape(4, 128, C).transpose(1, 0, 2).reshape(128, 4 * C)
    ).astype(bf16)
    reg0 = np.tile(_REG[0], (128, 1)).astype(np.float32)
    reg1 = np.tile(_REG[1], (128, 1)).astype(np.float32)
    pix = (np.arange(NB)[None, :] * 128 + np.arange(128)[:, None])
    ycoord = (pix // W).astype(np.float32)
    xcoord = (pix % W).astype(np.float32)
    shared = dict(
        wconv=wconv, bconv=bconv, dwt=dwt, reg0=reg0, reg1=reg1,
        ycoord=ycoord, xcoord=xcoord,
        gamma=inputs['gamma'].astype(np.float32),
        beta=inputs['beta'].astype(np.float32),
        rmean=inputs['rmean'].astype(np.float32),
        rvar=inputs['rvar'].astype(np.float32),
    )
    return shared


def kernel(**inputs):
    inputs = {k: np.asarray(v) for k, v in inputs.items()}
    if 'nc' not in _built:
        _built['nc'] = build_nc()
    nc = _built['nc']

    from concourse.bass_utils import run_bass_kernel_spmd
    shared = _host_pack(inputs)
    x = inputs['x'].astype(np.float32)
    in_maps = []
    for i in range(N):
        m = dict(shared)
        m['x'] = np.ascontiguousarray(x[i].reshape(C, HW))
        in_maps.append(m)
    res = run_bass_kernel_spmd(nc, in_maps, core_ids=list(range(N)))
    out = np.stack([res.results[i]['out'].reshape(C, H, W)
                    for i in range(N)])
    return out.astype(np.float32)


# revision 30
# speedup vs baseline: 1.0326x; 1.0326x over previous
"""Trainium2 Bass kernel for nn_AdaptBlockV2 (deformable-conv-v2 block).

Data-parallel over the batch axis: 8 samples -> 8 NeuronCores, one sample
per core. Inside each core:
  A) load x; build zero-padded CHW copy (bf16) for the convs; transpose x to
     HWC and write a "quad" gather table to DRAM (row r = channels of flat
     pixels [r, r+1, r+W, r+W+1], bf16) -- one indirect-DMA descriptor then
     fetches all 4 bilinear corners of one (pixel, tap).
  B) 15-channel 3x3 conv (offset transform T, translation tr, modulation
     mask) as 9 PSUM-accumulated matmuls; transpose conv output to
     pixel-major; bulk DVE math for sampling positions py/px, floor via
     floored-mod, corner weights (bilinear x mask x validity), and the flat
     gather index.
  C) per-slice pipeline: indirect DMA gather -> DVE weighted 4-corner
     combine -> PE transpose of samp to (tap,channel)-major -> matmul with
     dw -> BN (running stats) + residual + ReLU epilogue -> DMA out.

kernel(**inputs) takes FULL unsharded inputs, returns the FULL output.
"""
import numpy as np
import ml_dtypes

N, C, H, W = 8, 48, 96, 72
HW = H * W                       # 6912
LEAD = W + 2                     # 74: lead pad rows in the quad table
RQ = 7040                        # quad-table rows (55*128; >= HW+W+2)
QW = 256                         # quad-table row width (512B, dma_gather)
NB = HW // 128                   # 54 pixel blocks
QTOT = NB * 9                    # 486 (block, tap) chunks
PADW = W + 2                     # 74 padded conv row stride
PADLEN = (H + 2) * PADW          # 7252
BN_EPS = 1e-5
CONV_ROWS = 7                    # conv N-tile = 7 image rows = 504 pixels
SLICE_BLOCKS = 4                 # gather/combine slice = 4 pixel blocks

_REG = np.array([[-1, -1, -1, 0, 0, 0, 1, 1, 1],
                 [-1, 0, 1, -1, 0, 1, -1, 0, 1]], dtype=np.float32)

_built = {}


def _slices():
    out = []
    b = 0
    while b < NB:
        nb = min(SLICE_BLOCKS, NB - b)
        out.append((b, nb))
        b += nb
    return out


def build_nc(debug_taps=False, max_slices=None, stage='full'):
    import concourse.bass as bass
    import concourse.bacc as bacc
    import concourse.tile as tile
    from concourse import mybir
    from concourse.bass import IndirectOffsetOnAxis, AP
    from concourse.masks import make_identity
    from concourse.tile import add_dep_helper
    from contextlib import ExitStack

    dt = mybir.dt
    op = mybir.AluOpType
    act = mybir.ActivationFunctionType

    nc = bacc.Bacc("TRN2", target_bir_lowering=False, debug=False,
                   num_devices=N, dynamic_dma_scratch_size=16384)
    x_ext = nc.declare_dram_parameter("x", [C, HW], dt.float32, isOutput=False)
    wconv_ext = nc.declare_dram_parameter("wconv", [C, 135], dt.bfloat16, isOutput=False)
    bconv_ext = nc.declare_dram_parameter("bconv", [15], dt.float32, isOutput=False)
    dwt_ext = nc.declare_dram_parameter("dwt", [128, 192], dt.bfloat16, isOutput=False)
    reg0_ext = nc.declare_dram_parameter("reg0", [128, 9], dt.float32, isOutput=False)
    reg1_ext = nc.declare_dram_parameter("reg1", [128, 9], dt.float32, isOutput=False)
    yc_ext = nc.declare_dram_parameter("ycoord", [128, NB], dt.float32, isOutput=False)
    xc_ext = nc.declare_dram_parameter("xcoord", [128, NB], dt.float32, isOutput=False)
    gamma_ext = nc.declare_dram_parameter("gamma", [C], dt.float32, isOutput=False)
    beta_ext = nc.declare_dram_parameter("beta", [C], dt.float32, isOutput=False)
    rmean_ext = nc.declare_dram_parameter("rmean", [C], dt.float32, isOutput=False)
    rvar_ext = nc.declare_dram_parameter("rvar", [C], dt.float32, isOutput=False)
    out_ext = nc.declare_dram_parameter("out", [C, HW], dt.float32, isOutput=True)
    dbg = {}
    if debug_taps:
        for nm, shape, dty in (
                ("dbg_tcols", [128, NB * 15], dt.float32),
                ("dbg_idx", [128, QTOT], dt.int16),
                ("dbg_w4", [128, QTOT * 4], dt.bfloat16),
                ("dbg_gq", [128, SLICE_BLOCKS * 9 * QW], dt.bfloat16),
                ("dbg_samp", [128, SLICE_BLOCKS * 9 * C], dt.bfloat16),
                ("dbg_sampt", [128, 3 * 512], dt.bfloat16),
                ("dbg_hwc", [128, NB * C], dt.bfloat16),
                ("dbg_xq", [128, QW], dt.bfloat16),
                ("dbg_idxw", [128, QTOT * 8], dt.int16)):
            dbg[nm] = nc.declare_dram_parameter(nm, shape, dty, isOutput=True)

    x_quad = nc.dram_tensor("x_quad", [RQ, QW], dt.bfloat16)
    idx_dram = nc.dram_tensor("idx_dram", [128 * QTOT], dt.int16)

    with tile.TileContext(nc) as tc, ExitStack() as ctx:
        cp = ctx.enter_context(tc.tile_pool(name="const", bufs=1))
        tp = ctx.enter_context(tc.tile_pool(name="tmp", bufs=1))
        wp = ctx.enter_context(tc.tile_pool(name="work", bufs=2))
        pp_a = ctx.enter_context(tc.tile_pool(name="ps_a", bufs=2, space="PSUM"))
        pp_st = ctx.enter_context(tc.tile_pool(name="ps_st", bufs=2, space="PSUM"))
        pp_out = ctx.enter_context(tc.tile_pool(name="ps_out", bufs=2, space="PSUM"))

        # ---------------- constants / weights to SBUF ----------------
        x_sb = cp.tile([C, HW], dt.float32, tag="x_sb")
        nc.sync.dma_start(x_sb[:], x_ext[:])
        wconv_t = cp.tile([C, 135], dt.bfloat16, tag="wconv")
        nc.sync.dma_start(wconv_t[:], wconv_ext[:])
        bconv_t = cp.tile([15, 1], dt.float32, tag="bconv")
        nc.sync.dma_start(bconv_t[:], bconv_ext[:])
        dwt_t = cp.tile([128, 192], dt.bfloat16, tag="dwt")
        nc.sync.dma_start(dwt_t[:], dwt_ext[:])
        reg0_t = cp.tile([128, 9], dt.float32, tag="reg0")
        nc.sync.dma_start(reg0_t[:], reg0_ext[:])
        reg1_t = cp.tile([128, 9], dt.float32, tag="reg1")
        nc.sync.dma_start(reg1_t[:], reg1_ext[:])
        yc_t = cp.tile([128, NB], dt.float32, tag="yc")
        nc.sync.dma_start(yc_t[:], yc_ext[:])
        xc_t = cp.tile([128, NB], dt.float32, tag="xc")
        nc.sync.dma_start(xc_t[:], xc_ext[:])

        bn_in = {}
        for nm, ext in (("gamma", gamma_ext), ("beta", beta_ext),
                        ("rmean", rmean_ext), ("rvar", rvar_ext)):
            t = cp.tile([C, 1], dt.float32, tag=nm)
            nc.sync.dma_start(t[:], ext[:])
            bn_in[nm] = t

        id48 = cp.tile([C, C], dt.float32, tag="id48")
        make_identity(nc, id48[:])
        id16 = id48[0:15, 0:15]
        id128 = cp.tile([128, 128], dt.bfloat16, tag="id128")
        make_identity(nc, id128[:])

        # bn scale' = gamma * rsqrt(rvar+eps); shift' = beta - rmean*scale'
        veps = tp.tile([C, 1], dt.float32, tag="veps")
        nc.vector.tensor_scalar(veps[:], bn_in["rvar"][:], BN_EPS, None, op.add)
        vsq = tp.tile([C, 1], dt.float32, tag="vsq")
        nc.scalar.activation(vsq[:], veps[:], act.Sqrt)
        vri = tp.tile([C, 1], dt.float32, tag="vri")
        nc.vector.reciprocal(vri[:], vsq[:])
        scale_t = cp.tile([C, 1], dt.float32, tag="scale")
        nc.vector.tensor_tensor(scale_t[:], bn_in["gamma"][:], vri[:], op.mult)
        vms = tp.tile([C, 1], dt.float32, tag="vms")
        nc.vector.tensor_tensor(vms[:], bn_in["rmean"][:], scale_t[:], op.mult)
        shift_t = cp.tile([C, 1], dt.float32, tag="shift")
        nc.vector.tensor_tensor(shift_t[:], bn_in["beta"][:], vms[:], op.subtract)

        # ---------------- padded CHW copy (bf16) for convs ----------------
        x_pad = cp.tile([C, PADLEN], dt.bfloat16, tag="x_pad")
        nc.vector.memset(x_pad[:], 0.0)
        xpad_int = AP(x_pad.tensor, x_pad[:].offset + PADW + 1,
                      [x_pad[:].ap[0], [PADW, H], [1, W]])
        nc.vector.tensor_copy(xpad_int, x_sb[:])   # f32 -> bf16 cast on DVE

        # ---------------- x -> HWC (bf16) via PE transposes ----------------
        x_hwc = cp.tile([128, NB * C], dt.bfloat16, tag="x_hwc")
        for g in range((NB + 3) // 4):          # 4 blocks per PSUM tile
            nblk = min(4, NB - g * 4)
            ps = pp_a.tile([128, 4 * C], dt.float32, name="psA", tag="psA")
            for j in range(nblk):
                b = g * 4 + j
                nc.tensor.transpose(ps[:, j * C:(j + 1) * C],
                                    x_sb[:, b * 128:(b + 1) * 128], id48[:])
            nc.scalar.activation(x_hwc[:, g * 4 * C:(g * 4 + nblk) * C],
                                 ps[:, :nblk * C], act.Copy)

        # ---------------- quad table to DRAM ----------------
        # Zero the whole table (5 chained big writes), then write each slot
        # column j = x_hwc at row offset LEAD-shift_j. Issue chain keeps
        # order; gathers sem-wait on the last write only.
        zsrc = cp.tile([128, 1408], dt.bfloat16, tag="zsrc")
        nc.vector.memset(zsrc[:], 0.0)
        chain = []
        for zi in range(10):                     # 10 * 704 rows = 7040
            dst = AP(x_quad, zi * 704 * QW,
                     [[1408, 128], [1, 1408]])
            chain.append(nc.sync.dma_start(out=dst, in_=zsrc[:]))
        for j, shift in enumerate((0, 1, W, W + 1)):
            dst = AP(x_quad, (LEAD - shift) * QW + j * 64,
                     [[QW, 128], [128 * QW, NB], [1, C]])
            src = AP(x_hwc.tensor, x_hwc[:].offset,
                     [x_hwc[:].ap[0], [C, NB], [1, C]])
            chain.append(nc.sync.dma_start(out=dst, in_=src))
        for a, b in zip(chain[1:], chain[:-1]):
            add_dep_helper(a.ins, b.ins, sync=False,
                           reason="quad-table write chain")
        quad_writes = [chain[-1]]

        # ---------------- convs: 15ch 3x3 via 9 accumulated matmuls --------
        conv_sb = cp.tile([15, HW], dt.float32, tag="conv_sb")
        trows = [(t * CONV_ROWS, min(CONV_ROWS, H - t * CONV_ROWS))
                 for t in range((H + CONV_ROWS - 1) // CONV_ROWS)]
        for (r0, nr) in trows:
            psc = pp_a.tile([15, CONV_ROWS * W], dt.float32, name="psA2", tag="psA2")
            npx = nr * W
            for s in range(9):
                r, c = divmod(s, 3)
                rhs = AP(x_pad.tensor,
                         x_pad[:].offset + (r0 + r) * PADW + c,
                         [x_pad[:].ap[0], [PADW, nr], [1, W]])
                nc.tensor.matmul(psc[:, :npx], wconv_t[:, s * 15:(s + 1) * 15],
                                 rhs, start=(s == 0), stop=(s == 8))
            nc.scalar.activation(conv_sb[:, r0 * W:r0 * W + npx], psc[:, :npx],
                                 act.Identity, bias=bconv_t[:])

        # ---------------- conv output -> pixel-major (tcols) ---------------
        tcols = cp.tile([128, NB * 15], dt.float32, tag="tcols")
        for g in range((NB + 7) // 8):          # 8 blocks per PSUM tile
            nblk = min(8, NB - g * 8)
            ps = pp_a.tile([128, 8 * 15], dt.float32, name="psA", tag="psA")
            for j in range(nblk):
                b = g * 8 + j
                nc.tensor.transpose(ps[:, j * 15:(j + 1) * 15],
                                    conv_sb[:, b * 128:(b + 1) * 128],
                                    id16[:])
            nc.scalar.activation(tcols[:, g * 8 * 15:(g * 8 + nblk) * 15],
                                 ps[:, :nblk * 15], act.Copy)

        # ---------------- bulk offset / weight / index math ----------------
        def plane(tag):
            return tp.tile([128, QTOT], dt.float32, name=tag, tag=tag)

        def tcol_b(ch):     # tcols channel ch broadcast over taps [128,NB,9]
            return AP(tcols.tensor, tcols[:].offset + ch,
                      [tcols[:].ap[0], [15, NB], [0, 9]])

        def reg_b(t):       # REG row broadcast over blocks
            return AP(t.tensor, t[:].offset, [t[:].ap[0], [0, NB], [1, 9]])

        def coord_b(t):     # per-pixel coord broadcast over taps
            return AP(t.tensor, t[:].offset, [t[:].ap[0], [1, NB], [0, 9]])

        py = plane("py"); px = plane("px")
        t1 = plane("t1"); t2 = plane("t2"); t3 = plane("t3")
        fy = plane("fy"); fx = plane("fx")
        y0 = plane("y0"); x0 = plane("x0")
        wy0 = py; wy1 = px                       # reuse dead slots (disjoint cols)
        wx0 = plane("wx0"); wx1 = plane("wx1")
        idxf = fy                                # reuse dead slot
        cvt_i = tp.tile([128, QTOT], dt.int32, name="cvt_i", tag="cvt_i")
        w4 = cp.tile([128, QTOT * 4], dt.bfloat16, tag="w4")
        idx_t = cp.tile([128, QTOT], dt.int16, tag="idx")
        fold = cp.tile([16, 8 * QTOT], dt.int16, tag="fold")
        idxw = cp.tile([128, QTOT * 8], dt.int16, tag="idxw")

        # offset math emitted in groups of blocks so the first gathers can
        # start while later groups' index math is still running
        def emit_group(g0, gn):
            nq = gn * 9
            c0 = g0 * 9
            sl = slice(c0, c0 + nq)

            def tcol_b(ch):
                return AP(tcols.tensor, tcols[:].offset + ch + g0 * 15,
                          [tcols[:].ap[0], [15, gn], [0, 9]])

            def reg_b(t):
                return AP(t.tensor, t[:].offset, [t[:].ap[0], [0, gn], [1, 9]])

            def coord_b(t):
                return AP(t.tensor, t[:].offset + g0,
                          [t[:].ap[0], [1, gn], [0, 9]])

            tt = nc.vector.tensor_tensor
            ts = nc.vector.tensor_scalar
            tt(t1[:, sl], reg_b(reg0_t), tcol_b(0), op.mult)
            tt(t2[:, sl], reg_b(reg1_t), tcol_b(1), op.mult)
            tt(t3[:, sl], t1[:, sl], t2[:, sl], op.add)
            tt(t1[:, sl], t3[:, sl], tcol_b(4), op.add)
            tt(py[:, sl], t1[:, sl], coord_b(yc_t), op.add)
            tt(t1[:, sl], reg_b(reg0_t), tcol_b(2), op.mult)
            tt(t2[:, sl], reg_b(reg1_t), tcol_b(3), op.mult)
            tt(t3[:, sl], t1[:, sl], t2[:, sl], op.add)
            tt(t1[:, sl], t3[:, sl], tcol_b(5), op.add)
            tt(px[:, sl], t1[:, sl], coord_b(xc_t), op.add)

            # floor(v) = int(v) - (v < int(v)); rounding-mode agnostic
            for (v, fl, fr) in ((py, y0, fy), (px, x0, fx)):
                nc.vector.tensor_copy(cvt_i[:, sl], v[:, sl])
                nc.vector.tensor_copy(t1[:, sl], cvt_i[:, sl])
                tt(t2[:, sl], v[:, sl], t1[:, sl], op.is_lt)
                tt(fl[:, sl], t1[:, sl], t2[:, sl], op.subtract)
                tt(fr[:, sl], v[:, sl], fl[:, sl], op.subtract)

            mk_b = AP(tcols.tensor, tcols[:].offset + 6 + g0 * 15,
                      [tcols[:].ap[0], [15, gn], [1, 9]])
            ts(t1[:, sl], y0[:, sl], 0.0, None, op.is_ge)
            ts(t2[:, sl], y0[:, sl], float(H - 1), None, op.is_le)
            tt(t3[:, sl], t1[:, sl], t2[:, sl], op.mult)
            ts(t1[:, sl], fy[:, sl], -1.0, 1.0, op.mult, op.add)
            tt(wy0[:, sl], t1[:, sl], t3[:, sl], op.mult)
            ts(t1[:, sl], y0[:, sl], -1.0, None, op.is_ge)
            ts(t2[:, sl], y0[:, sl], float(H - 2), None, op.is_le)
            tt(t3[:, sl], t1[:, sl], t2[:, sl], op.mult)
            tt(wy1[:, sl], fy[:, sl], t3[:, sl], op.mult)
            ts(t1[:, sl], x0[:, sl], 0.0, None, op.is_ge)
            ts(t2[:, sl], x0[:, sl], float(W - 1), None, op.is_le)
            tt(t3[:, sl], t1[:, sl], t2[:, sl], op.mult)
            ts(t1[:, sl], fx[:, sl], -1.0, 1.0, op.mult, op.add)
            tt(wx0[:, sl], t1[:, sl], t3[:, sl], op.mult)
            ts(t1[:, sl], x0[:, sl], -1.0, None, op.is_ge)
            ts(t2[:, sl], x0[:, sl], float(W - 2), None, op.is_le)
            tt(t3[:, sl], t1[:, sl], t2[:, sl], op.mult)
            tt(wx1[:, sl], fx[:, sl], t3[:, sl], op.mult)
            tt(wy0[:, sl], wy0[:, sl], mk_b, op.mult)
            tt(wy1[:, sl], wy1[:, sl], mk_b, op.mult)

            def w4_slot(j):
                return AP(w4.tensor, w4[:].offset + j + c0 * 4,
                          [w4[:].ap[0], [4, nq]])
            tt(w4_slot(0), wy0[:, sl], wx0[:, sl], op.mult)
            tt(w4_slot(1), wy0[:, sl], wx1[:, sl], op.mult)
            tt(w4_slot(2), wy1[:, sl], wx0[:, sl], op.mult)
            tt(w4_slot(3), wy1[:, sl], wx1[:, sl], op.mult)

            # gather index = clip(LEAD + y0*W + x0, 0, HW+W+1), int16
            nc.vector.scalar_tensor_tensor(idxf[:, sl], y0[:, sl], float(W),
                                           x0[:, sl], op.mult, op.add)
            ts(idxf[:, sl], idxf[:, sl], float(LEAD), 0.0, op.add, op.max)
            ts(idxf[:, sl], idxf[:, sl], float(HW + W + 1), None, op.min)
            nc.vector.tensor_copy(idx_t[:, sl], idxf[:, sl])

            # 16-wrap via DRAM round-trip fold + in-partition interleave
            wr = nc.sync.dma_start(
                out=AP(idx_dram, c0, [[QTOT, 128], [1, nq]]),
                in_=idx_t[:, sl])
            rdf = nc.sync.dma_start(
                out=AP(fold.tensor, fold[:].offset + c0,
                       [fold[:].ap[0], [QTOT, 8], [1, nq]]),
                in_=AP(idx_dram, c0, [[QTOT, 16], [16 * QTOT, 8], [1, nq]]))
            add_dep_helper(rdf.ins, wr.ins, sync=False, reason="idx fold rt")
            i16 = idxw[0:16, :]
            nc.vector.tensor_copy(
                AP(i16.tensor, i16.offset + c0 * 8,
                   [i16.ap[0], [1, 8], [8, nq]]),
                AP(fold.tensor, fold[:].offset + c0,
                   [fold[:].ap[0], [QTOT, 8], [1, nq]]))
            for g in range(1, 8):
                nc.sync.dma_start(
                    out=idxw[16 * g:16 * (g + 1), c0 * 8:(c0 + nq) * 8],
                    in_=idxw[0:16, c0 * 8:(c0 + nq) * 8])

        for (g0, gn) in ((0, 18), (18, 18), (36, 18)):
            emit_group(g0, gn)

        # ---------------- gather / combine / matmul / epilogue -------------
        if debug_taps:
            nc.sync.dma_start(dbg["dbg_tcols"][:], tcols[:])
            nc.sync.dma_start(dbg["dbg_idx"][:], idx_t[:])
            nc.sync.dma_start(dbg["dbg_w4"][:], w4[:])
            nc.sync.dma_start(dbg["dbg_hwc"][:], x_hwc[:])
            xqs = cp.tile([128, QW], dt.bfloat16, tag="xqs")
            rd = nc.sync.dma_start(out=xqs[:], in_=x_quad[100:228, :])
            add_dep_helper(rd.ins, quad_writes[0].ins,
                           reason="dbg read after quad build")
            nc.sync.dma_start(dbg["dbg_xq"][:], xqs[:])
            wdep = nc.sync.dma_start(dbg["dbg_idxw"][:], idxw[:])

        slist = _slices() if max_slices is None else _slices()[:max_slices]
        for si, (b0, nb) in enumerate(slist):
            Q = nb * 9
            npx = nb * 128
            q0 = b0 * 9
            gq = wp.tile([128, SLICE_BLOCKS * 9 * QW], dt.bfloat16, tag="gq",
                         bufs=3)
            gq3 = AP(gq.tensor, gq[:].offset,
                     [gq[:].ap[0], [QW, Q], [1, QW]])
            gi = nc.gpsimd.dma_gather(
                out_ap=gq3,
                in_ap=x_quad[:],
                idxs_ap=idxw[:, q0 * 8:(q0 + Q) * 8],
                num_idxs=Q * 128,
                num_idxs_reg=Q * 128,
                elem_size=QW,
                single_packet=False,
            )
            for qw in quad_writes:
                add_dep_helper(gi.ins, qw.ins, reason="gather after quad build")

            if debug_taps and si == 0:
                nc.sync.dma_start(dbg["dbg_gq"][:], gq[:])

            if stage == 'gather':
                continue
            # weighted 4-corner combine (in-place over the gathered tile;
            # slot pads multiply garbage that is never read)
            prod = gq
            gview = gq[:, :Q * QW]
            wb = AP(w4.tensor, w4[:].offset + q0 * 4,
                    [w4[:].ap[0], [1, Q * 4], [0, 64]])
            nc.vector.tensor_tensor(gview, gview, wb, op.mult)

            def pview(off):
                return AP(prod.tensor, prod[:].offset + off,
                          [prod[:].ap[0], [QW, Q], [1, C]])
            sa = wp.tile([128, SLICE_BLOCKS * 9 * C], dt.bfloat16, tag="sa")
            sb2 = wp.tile([128, SLICE_BLOCKS * 9 * C], dt.bfloat16, tag="sb2")
            nc.vector.tensor_tensor(sa[:, :Q * C], pview(0), pview(64), op.add)
            nc.vector.tensor_tensor(sb2[:, :Q * C], pview(128), pview(192),
                                    op.add)
            samp = sa
            nc.vector.tensor_tensor(samp[:, :Q * C], sa[:, :Q * C],
                                    sb2[:, :Q * C], op.add)

            if debug_taps and si == 0:
                nc.sync.dma_start(dbg["dbg_samp"][:], samp[:])

            if stage == 'combine':
                continue
            # transpose samp [128px, 432] -> sampt [(tap,ch) 128-chunks, px]
            sampt = wp.tile([128, 4 * SLICE_BLOCKS * 128], dt.bfloat16, tag="sampt")
            for ib in range(nb):
                ps = pp_st.tile([128, 512], dt.bfloat16, tag="ps_st")
                base = ib * 9 * C
                for ch in range(3):
                    nc.tensor.transpose(
                        ps[:, ch * 128:(ch + 1) * 128],
                        samp[:, base + ch * 128: base + (ch + 1) * 128],
                        id128[:])
                nc.tensor.transpose(ps[0:C, 384:512],
                                    samp[:, base + 384: base + 432], id128[:])
                dst02 = AP(sampt.tensor, sampt[:].offset + ib * 128,
                           [sampt[:].ap[0], [npx, 3], [1, 128]])
                nc.scalar.activation(dst02, ps[:, 0:384], act.Copy)
                nc.scalar.activation(sampt[0:C, 3 * npx + ib * 128:
                                           3 * npx + (ib + 1) * 128],
                                     ps[0:C, 384:512], act.Copy)

            if debug_taps and si == 0:
                nc.sync.dma_start(dbg["dbg_sampt"][:], sampt[:, 0:3 * 512])

            if stage == 'transpose':
                continue
            # matmul: out[o, px] += dwT_chunk.T @ sampt_chunk
            pso = pp_out.tile([C, SLICE_BLOCKS * 128], dt.float32, tag="ps_out")
            for ch in range(3):
                nc.tensor.matmul(pso[:, :npx], dwt_t[:, ch * C:(ch + 1) * C],
                                 sampt[:, ch * npx:(ch + 1) * npx],
                                 start=(ch == 0), stop=False)
            nc.tensor.matmul(pso[:, :npx], dwt_t[0:C, 3 * C:4 * C],
                             sampt[0:C, 3 * npx:3 * npx + npx],
                             start=False, stop=True)

            # epilogue: BN(running stats) + residual + relu -> DRAM
            bno = wp.tile([C, SLICE_BLOCKS * 128], dt.float32, tag="bno")
            nc.scalar.activation(bno[:, :npx], pso[:, :npx], act.Identity,
                                 bias=shift_t[:], scale=scale_t[:])
            nc.vector.tensor_tensor(bno[:, :npx], bno[:, :npx],
                                    x_sb[:, b0 * 128:b0 * 128 + npx], op.add)
            nc.vector.tensor_scalar(bno[:, :npx], bno[:, :npx], 0.0, None,
                                    op.max)
            nc.sync.dma_start(out_ext[:, b0 * 128:b0 * 128 + npx],
                              bno[:, :npx])

    nc.compile()
    return nc


def _host_pack(inputs):
    """Weight/constant layout prep (no input-data compute)."""
    bf16 = ml_dtypes.bfloat16
    wa = np.concatenate([inputs['tm_w'], inputs['tr_w'], inputs['mk_w']],
                        axis=0)                       # [15, C, 3, 3]
    wconv = np.ascontiguousarray(
        wa.transpose(1, 2, 3, 0).reshape(C, 9 * 15)).astype(bf16)
    bconv = np.concatenate([inputs['tm_b'], inputs['tr_b'],
                            inputs['mk_b']]).astype(np.float32)
    dwr = inputs['dw'].reshape(C, C, 9)               # [o, c, k]
    dwT = np.zeros((512, C), dtype=np.float32)
    dwT[:432] = dwr.transpose(2, 1, 0).reshape(9 * C, C)
    dwt = np.ascontiguousarray(
        dwT.reshape(4, 128, C).transpose(1, 0, 2).reshape(128, 4 * C)
    ).astype(bf16)
    reg0 = np.tile(_REG[0], (128, 1)).astype(np.float32)
    reg1 = np.tile(_REG[1], (128, 1)).astype(np.float32)
    pix = (np.arange(NB)[None, :] * 128 + np.arange(128)[:, None])
    ycoord = (pix // W).astype(np.float32)
    xcoord = (pix % W).astype(np.float32)
    shared = dict(
        wconv=wconv, bconv=bconv, dwt=dwt, reg0=reg0, reg1=reg1,
        ycoord=ycoord, xcoord=xcoord,
        gamma=inputs['gamma'].astype(np.float32),
        beta=inputs['beta'].astype(np.float32),
        rmean=inputs['rmean'].astype(np.float32),
        rvar=inputs['rvar'].astype(np.float32),
    )
    return shared


def kernel(**inputs):
    inputs = {k: np.asarray(v) for k, v in inputs.items()}
    if 'nc' not in _built:
        _built['nc'] = build_nc()
    nc = _built['nc']

    from concourse.bass_utils import run_bass_kernel_spmd
    shared = _host_pack(inputs)
    x = inputs['x'].astype(np.float32)
    in_maps = []
    for i in range(N):
        m = dict(shared)
        m['x'] = np.ascontiguousarray(x[i].reshape(C, HW))
        in_maps.append(m)
    res = run_bass_kernel_spmd(nc, in_maps, core_ids=list(range(N)))
    out = np.stack([res.results[i]['out'].reshape(C, H, W)
                    for i in range(N)])
    return out.astype(np.float32)
